# revision 22
# baseline (speedup 1.0000x reference)
"""MixedArityTreeLSTM Trainium2 kernel.

Level-synchronous bottom-up Tree-LSTM over B=256 heap-indexed perfect binary
trees (511 nodes, depth 8), E=H=128. Pure data-parallel over 8 NeuronCores
(32 trees per core); all weights replicated.

v4: host packs token-dependent data as dense feature-major bf16 streams (pure
data movement; arithmetic stays on device or is a vocab-indexed weight-only
table transform): hleaf = rows of tanh(emb @ W3 + bW3); xT = emb rows.

Every level is stored in BIT-REVERSED node order ("parity layout"), trees
fastest: left children of a level's positions [a, b) sit at the child level's
positions [a, b) and right children at [HALF + a, HALF + b). All child-pair
views are contiguous, which keeps the DVE on its fast 16-bit path.

Device per internal level (feature-major [H(part), nodes(free)]):
    pre_g = W_g^T x + Ubt_g^T (m*h_l) + Ubb_g^T (m*h_r) + Uun_g^T ((1-m)*h_l)
            + [b_g; d_g]^T [ones; m]      (K=2 matmul: bias + arity delta)
Gate pairs share PSUM tiles so one activation covers two gates. The unary
f_r kill uses cob = m * c_r. All elementwise work is bf16 in SBUF; c is bf16
except the root level. Emission is software-pipelined (masks/body/chain) and
levels <=4 split into two position-halves that ping-pong.
"""

import numpy as np
import ml_dtypes

B, D = 256, 8
V, E, H = 32000, 128, 128
NCORES = 8
BL = B // NCORES  # 32 trees per core

LVL_N = {l: BL * (2**l) for l in range(D + 1)}
INT_LEVELS = list(range(D - 1, -1, -1))  # 7..0
LVL_OFF = {}
_off = 0
for _l in INT_LEVELS:
    LVL_OFF[_l] = _off
    _off += LVL_N[_l]
XCOLS = _off  # 8160
LEAF_COLS = LVL_N[D]  # 8192

# bit-reversal position->node order per level: sig[l][i] = node at position i
SIG = {0: np.array([0])}
for _l in range(1, D + 1):
    SIG[_l] = np.concatenate([2 * SIG[_l - 1], 2 * SIG[_l - 1] + 1])

CPL = {8: 16, 7: 8, 6: 4, 5: 2, 4: 1, 3: 2, 2: 1, 1: 1, 0: 1}
CW = {l: LVL_N[l] // CPL[l] for l in range(D + 1)}

BIG_SEQ = [
    (7, 0), (7, 4), (6, 0), (7, 1), (7, 5), (6, 1), (7, 2),
    (7, 6), (6, 2), (5, 0), (7, 3), (7, 7), (6, 3), (5, 1),
]
TAIL_SEQ = [(l, j) for l in (4, 3, 2, 1, 0) for j in range(CPL[l])]
SEQ = BIG_SEQ + TAIL_SEQ


def _children(lvl, j):
    """Child chunks (lvl+1, jj) whose h/c this chunk consumes (parity layout)."""
    if lvl == D - 1:
        return []  # children are leaves (DMA'd, no chain)
    N = CW[lvl]
    c0 = j * N
    half = LVL_N[lvl]
    spans = [(c0, c0 + N), (half + c0, half + c0 + N)]
    out = []
    for jj in range(CPL[lvl + 1]):
        a, b = jj * CW[lvl + 1], (jj + 1) * CW[lvl + 1]
        if any(a < hi and b > lo for lo, hi in spans) and (lvl + 1, jj) not in out:
            out.append((lvl + 1, jj))
    return out


BF16 = ml_dtypes.bfloat16

_CACHE = {}


def _build_nc():
    if "nc" in _CACHE:
        return _CACHE["nc"]

    from contextlib import ExitStack

    import concourse.mybir as mybir
    import concourse.tile as tile
    from concourse import bacc

    dt = mybir.dt
    AF = mybir.ActivationFunctionType

    nc = bacc.Bacc()

    hleaf_d = nc.dram_tensor("hleaf", [128, LEAF_COLS], dt.bfloat16, kind="ExternalInput")
    xall_d = nc.dram_tensor("xall", [128, XCOLS], dt.bfloat16, kind="ExternalInput")
    mbr_d = nc.dram_tensor("maskb", [1, XCOLS], dt.bfloat16, kind="ExternalInput")
    mk2_d = nc.dram_tensor("mk2", [2, XCOLS], dt.bfloat16, kind="ExternalInput")
    # host-pretransposed weights: [E, gate, H] etc., contiguous uploads
    wt_d = nc.dram_tensor("wt_bf", [E, 4 * H], dt.bfloat16, kind="ExternalInput")
    ubtt_d = nc.dram_tensor("ubtt_bf", [H, 5 * H], dt.bfloat16, kind="ExternalInput")
    ubbt_d = nc.dram_tensor("ubbt_bf", [H, 5 * H], dt.bfloat16, kind="ExternalInput")
    uunt_d = nc.dram_tensor("uunt_bf", [H, 4 * H], dt.bfloat16, kind="ExternalInput")
    bd_d = nc.dram_tensor("bd_bf", [2, 5 * H], dt.bfloat16, kind="ExternalInput")

    h_out_d = nc.dram_tensor("h_out", [H, BL], dt.float32, kind="ExternalOutput")
    c_out_d = nc.dram_tensor("c_out", [H, BL], dt.float32, kind="ExternalOutput")

    with tile.TileContext(nc) as tc, ExitStack() as ctx:
        consts = ctx.enter_context(tc.tile_pool(name="consts", bufs=1))
        lev = ctx.enter_context(tc.tile_pool(name="lev", bufs=1))

        h_t = {}
        c_t = {}
        h_t[D] = lev.tile([H, LEAF_COLS], dt.bfloat16, name="h_leaf", tag="h_leaf")
        for lvl in INT_LEVELS:
            n = LVL_N[lvl]
            hdt = dt.float32 if lvl == 0 else dt.bfloat16
            h_t[lvl] = lev.tile([H, n], hdt, name=f"h_l{lvl}", tag=f"h_l{lvl}")
            c_t[lvl] = lev.tile([H, n], hdt, name=f"c_l{lvl}", tag=f"c_l{lvl}")

        xt = lev.tile([128, XCOLS], dt.bfloat16, name="xT", tag="xT")
        mbc_sb = lev.tile([128, XCOLS], dt.bfloat16, name="mbc", tag="mbc")
        mk2_sb = lev.tile([2, XCOLS], dt.bfloat16, name="mk2", tag="mk2")

        # SP queue: leaf h (alternating halves: chunk (7,j) needs leaf j and
        # 8+j) interleaved with x pieces, in first-need order.
        def sdma(tile_, dram, a, b):
            nc.sync.dma_start(out=tile_[:, a:b], in_=dram[:, a:b])

        def xdma(a, b):
            nc.sync.dma_start(out=xt[:, a:b], in_=xall_d[:, a:b])

        # x first: the hoisted W-pass runway consumes xs before leaf h
        xdma(0, 1024)                       # (7,0)/(7,1) x
        sdma(h_t[D], hleaf_d, 0, 1024)      # (7,0)/(7,1) left
        sdma(h_t[D], hleaf_d, 4096, 5120)   # (7,0)/(7,1) right
        xdma(2048, 3072)                    # (7,4)/(7,5) x
        sdma(h_t[D], hleaf_d, 2048, 3072)   # (7,4)/(7,5) left
        sdma(h_t[D], hleaf_d, 6144, 7168)   # (7,4)/(7,5) right
        xdma(1024, 2048)                    # (7,2)/(7,3) x
        sdma(h_t[D], hleaf_d, 1024, 2048)   # (7,2)/(7,3) left
        sdma(h_t[D], hleaf_d, 5120, 6144)   # (7,2)/(7,3) right
        xdma(3072, 4096)                    # (7,6)/(7,7) x
        sdma(h_t[D], hleaf_d, 3072, 4096)   # (7,6)/(7,7) left
        sdma(h_t[D], hleaf_d, 7168, 8192)   # (7,6)/(7,7) right
        xdma(4096, 6144)                    # L6 x
        xdma(6144, XCOLS)                   # L5 + tail x

        # Pool (gpsimd) queue: load the mask row (16KB) and broadcast it
        # across partitions on-device (keeps 2MB off the HBM startup burst)
        mbrow_sb = lev.tile([1, XCOLS], dt.bfloat16, name="mbrow", tag="mbrow")
        nc.gpsimd.dma_start(out=mbrow_sb[:, :], in_=mbr_d[:, :])
        # pieces in chunk-need order: L7 bodies touch 0/2048/512/...,
        # L6 bodies 4096+, L5 6144+, tail last
        for a, b in (
            (0, 512), (2048, 2560), (4096, 4608),
            (512, 1024), (2560, 3072), (4608, 5120),
            (1024, 1536), (3072, 3584), (5120, 5632),
            (1536, 2048), (3584, 4096), (5632, 6144),
            (6144, 7168), (7168, XCOLS),
        ):
            nc.gpsimd.partition_broadcast(mbc_sb[:, a:b], mbrow_sb[:, a:b])

        # Act queue: weights (host-pretransposed, contiguous)
        w_sb = consts.tile([E, 4, H], dt.bfloat16)
        nc.scalar.dma_start(
            out=w_sb, in_=wt_d[:, :].rearrange("e (g h) -> e g h", g=4)
        )
        nc.scalar.dma_start(out=mk2_sb[:, 0:4096], in_=mk2_d[:, 0:4096])
        bd_sb = consts.tile([2, 5, H], dt.bfloat16)
        nc.scalar.dma_start(
            out=bd_sb, in_=bd_d[:, :].rearrange("k (g h) -> k g h", g=5)
        )
        nc.scalar.dma_start(out=mk2_sb[:, 4096:XCOLS], in_=mk2_d[:, 4096:XCOLS])
        ubt_sb = consts.tile([H, 5, H], dt.bfloat16)
        nc.scalar.dma_start(
            out=ubt_sb, in_=ubtt_d[:, :].rearrange("k (g h) -> k g h", g=5)
        )
        ubb_sb = consts.tile([H, 5, H], dt.bfloat16)
        nc.scalar.dma_start(
            out=ubb_sb, in_=ubbt_d[:, :].rearrange("k (g h) -> k g h", g=5)
        )
        uun_sb = consts.tile([H, 4, H], dt.bfloat16)
        nc.scalar.dma_start(
            out=uun_sb, in_=uunt_d[:, :].rearrange("k (g h) -> k g h", g=4)
        )

        # PSUM: big tags (5 banks) + tail-odd parity tags (3 banks) = 8 banks
        psum = ctx.enter_context(tc.tile_pool(name="psum", bufs=1, space="PSUM"))
        work = ctx.enter_context(tc.tile_pool(name="work", bufs=4))

        # PE p-state warmup: ~3.5us of dummy matmuls so the Tensor engine is
        # at max clock when the first real chunk issues. Reuses the tail-odd
        # psum tag (idle until the tail) and only depends on the w_sb upload.
        wflat = w_sb.rearrange("e g h -> e (g h)")
        warm = psum.tile([H, 512], dt.float32, tag="tU1", name="warm")
        for _ in range(16):
            nc.tensor.matmul(warm, w_sb[:, 0, :], wflat, start=True, stop=True)

        state = {}

        def phase_masks(lvl, j):
            N = CW[lvl]
            c0 = j * N
            half = LVL_N[lvl]
            moff = LVL_OFF[lvl] + c0
            top = lvl == D - 1
            hch = h_t[lvl + 1]
            h_e = hch[:, c0 : c0 + N]
            h_o = hch[:, half + c0 : half + c0 + N]
            mb = mbc_sb[:, moff : moff + N]

            heb = work.tile([128, N], dt.bfloat16, tag="heb", name="heb")
            nc.vector.tensor_mul(heb, h_e, mb)
            hob = work.tile([128, N], dt.bfloat16, tag="hob", name="hob")
            nc.vector.tensor_mul(hob, h_o, mb)
            heu = work.tile([128, N], dt.bfloat16, tag="heu", name="heu")
            nc.vector.tensor_sub(heu, h_e, heb)
            st = {"heb": heb, "hob": hob, "heu": heu}
            if not top:
                cch = c_t[lvl + 1]
                st["c_e"] = cch[:, c0 : c0 + N]
                cob = work.tile([128, N], dt.bfloat16, tag="cob", name="cob")
                nc.vector.tensor_mul(cob, cch[:, half + c0 : half + c0 + N], mb)
                st["cob"] = cob
            state[(lvl, j)] = st

        def phase_body(lvl, j):
            N = CW[lvl]
            c0 = j * N
            moff = LVL_OFF[lvl] + c0
            top = lvl == D - 1
            st = state[(lvl, j)]
            heb, hob, heu = st["heb"], st["hob"], st["heu"]
            xs = xt[:, moff : moff + N]
            mk = mk2_sb[:, moff : moff + N]
            odd = lvl == 3 and j == 1
            tagA = "tA1" if odd else "bgA"
            tagB = "tB1" if odd else "bgB"
            tagU = "tU1" if odd else "bgu"

            # gate table: (psum slice key, W idx, Ubin idx, Uun idx, bd idx)
            # u first: the chain's first op t1 = gi*gu consumes gu, so the
            # tanh act must not be last; (fr, o) close last (t3/hmul inputs)
            if top:
                gates = [("U", 3, 4, 3, 4), ("A0", 0, 0, 0, 0), ("A1", 2, 3, 2, 3)]
            else:
                gates = [
                    ("U", 3, 4, 3, 4),     # u
                    ("A0", 0, 0, 0, 0),    # i
                    ("A1", 1, 1, 1, 1),    # fl
                    ("B0", 1, 2, None, 2),  # fr
                    ("B1", 2, 3, 2, 3),    # o
                ]

            pA = psum.tile([H, 2 * N], dt.float32, tag=tagA, name=f"pA{lvl}_{j}")
            pB = (
                psum.tile([H, 2 * N], dt.float32, tag=tagB, name=f"pB{lvl}_{j}")
                if not top
                else None
            )
            pU = psum.tile([H, N], dt.float32, tag=tagU, name=f"pU{lvl}_{j}")
            sl = {
                "A0": pA[:, 0:N],
                "A1": pA[:, N : 2 * N],
                "U": pU,
            }
            if not top:
                sl["B0"] = pB[:, 0:N]
                sl["B1"] = pB[:, N : 2 * N]

            # Big chunks (bank-aligned slices): dependency-free runway first —
            # W*x and bias/delta passes need only xs/mk, absorbing the
            # previous level's chain latency. Tail chunks share PSUM zero
            # regions between gate slices, so they run gate-major (one open
            # accumulation group per region).
            hoist = N >= 512
            if hoist:
                for key, wi, ubi, uui, gi_ in gates:
                    nc.tensor.matmul(
                        sl[key], w_sb[:, wi, :], xs, start=True, stop=False
                    )
                for key, wi, ubi, uui, gi_ in gates:
                    nc.tensor.matmul(
                        sl[key], bd_sb[:, gi_, :], mk, start=False, stop=False
                    )
            for key, wi, ubi, uui, gi_ in gates:
                ps = sl[key]
                if not hoist:
                    nc.tensor.matmul(ps, w_sb[:, wi, :], xs, start=True, stop=False)
                    nc.tensor.matmul(
                        ps, bd_sb[:, gi_, :], mk, start=False, stop=False
                    )
                nc.tensor.matmul(ps, ubt_sb[:, ubi, :], heb, start=False, stop=False)
                nc.tensor.matmul(
                    ps, ubb_sb[:, ubi, :], hob, start=False, stop=uui is None
                )
                if uui is not None:
                    nc.tensor.matmul(ps, uun_sb[:, uui, :], heu, start=False, stop=True)
                if key == "A1":
                    gAB = work.tile([128, 2 * N], dt.bfloat16, tag="gAB", name="gAB")
                    nc.scalar.activation(gAB, pA, AF.Sigmoid)
                    if top:
                        st["gi"], st["go"] = gAB[:, 0:N], gAB[:, N : 2 * N]
                    else:
                        st["gi"], st["gfl"] = gAB[:, 0:N], gAB[:, N : 2 * N]
                elif key == "B1":
                    gFO = work.tile([128, 2 * N], dt.bfloat16, tag="gFO", name="gFO")
                    nc.scalar.activation(gFO, pB, AF.Sigmoid)
                    st["gfr"], st["go"] = gFO[:, 0:N], gFO[:, N : 2 * N]
                elif key == "U":
                    gu = work.tile([128, N], dt.bfloat16, tag="gu", name="gu")
                    nc.scalar.activation(gu, pU, AF.Tanh)
                    st["gu"] = gu

        def phase_chain(lvl, j):
            N = CW[lvl]
            c0 = j * N
            top = lvl == D - 1
            st = state.pop((lvl, j))
            cs = c_t[lvl][:, c0 : c0 + N]
            wdt = dt.float32 if lvl == 0 else dt.bfloat16
            if top:
                nc.vector.tensor_mul(cs, st["gi"], st["gu"])
            else:
                t1 = work.tile([128, N], wdt, tag="t1", name="t1")
                nc.vector.tensor_mul(t1, st["gi"], st["gu"])
                t2 = work.tile([128, N], wdt, tag="t2", name="t2")
                nc.vector.tensor_mul(t2, st["gfl"], st["c_e"])
                nc.vector.tensor_add(cs, t1, t2)
                t3 = work.tile([128, N], wdt, tag="t3", name="t3")
                nc.vector.tensor_mul(t3, st["gfr"], st["cob"])
                nc.vector.tensor_add(cs, cs, t3)
            tch = work.tile([128, N], wdt, tag="tch", name="tch")
            nc.scalar.activation(tch, cs, AF.Tanh)
            nc.vector.tensor_mul(h_t[lvl][:, c0 : c0 + N], st["go"], tch)

        pending = []
        for lvl, j in SEQ:
            for ch in _children(lvl, j):
                if ch in pending:
                    phase_chain(*ch)
                    pending.remove(ch)
            phase_masks(lvl, j)
            phase_body(lvl, j)
            pending.append((lvl, j))
            while len(pending) > 1:
                phase_chain(*pending.pop(0))
        for ch in pending:
            phase_chain(*ch)

        nc.sync.dma_start(out=h_out_d[:, :], in_=h_t[0][:, :BL])
        nc.sync.dma_start(out=c_out_d[:, :], in_=c_t[0][:, :BL])

    nc.finalize()
    _CACHE["nc"] = nc
    return nc


def prep_core_inputs(tokens_c, arity_c, shared):
    """Per-core input map: gather rows of precomputed vocab tables, pack masks.

    Each level is packed in bit-reversed node order, trees fastest
    (col = position * BL + tree).
    """
    tokens_c = np.asarray(tokens_c)
    arity_c = np.asarray(arity_c, np.int32)
    emb_bf = shared["_emb_bf"]
    hleaf_tab = shared["_hleaf_tab"]

    leaf_toks = tokens_c[:, (2**D - 1) + SIG[D]].T.reshape(-1)  # [8192] node-major
    hleaf = np.ascontiguousarray(hleaf_tab[leaf_toks].T)

    xcols = []
    mcols = []
    for lvl in INT_LEVELS:
        off = 2**lvl - 1
        toks = tokens_c[:, off + SIG[lvl]].T.reshape(-1)
        xcols.append(emb_bf[toks].T)
        mcols.append(
            (arity_c[:, off + SIG[lvl]].T.reshape(-1) == 1).astype(BF16)
        )
    xall = np.ascontiguousarray(np.concatenate(xcols, axis=1))  # [128, 8160]
    maskb = np.concatenate(mcols)[None, :]  # [1, 8160]
    mk2 = np.concatenate([np.ones_like(maskb), maskb], axis=0)

    out = {k: v for k, v in shared.items() if not k.startswith("_")}
    out.update(hleaf=hleaf, xall=xall, mk2=mk2, maskb=maskb)
    return out


def prep_shared_inputs(emb, W, bW, Ubin, bUbin, Uun, bUun):
    emb = np.asarray(emb, np.float32)
    W = np.asarray(W, np.float32)
    bW = np.asarray(bW, np.float32)
    Ubin = np.asarray(Ubin, np.float32)
    bUbin = np.asarray(bUbin, np.float32)
    Uun = np.asarray(Uun, np.float32)
    bUun = np.asarray(bUun, np.float32)

    b_rows = np.stack(
        [
            bW[0] + bUun[0],      # i
            bW[1] + bUun[1],      # fl
            bW[1] + bUbin[2],     # fr (binary value; unary killed via cob)
            bW[2] + bUun[2],      # o
            bW[3] + bUun[3],      # u
        ]
    )
    d_rows = np.stack(
        [
            bUbin[0] - bUun[0],
            bUbin[1] - bUun[1],
            np.zeros(H, np.float32),
            bUbin[3] - bUun[2],
            bUbin[4] - bUun[3],
        ]
    )

    emb_bf = emb.astype(BF16)
    hleaf_tab = np.tanh(emb @ W[3] + bW[3]).astype(BF16)

    def t_pack(a):  # [G, K, H] -> [K, G*H] contiguous
        return np.ascontiguousarray(np.transpose(a, (1, 0, 2)).reshape(a.shape[1], -1))

    return dict(
        _emb_bf=emb_bf,
        _hleaf_tab=hleaf_tab,
        wt_bf=t_pack(W.astype(BF16)),
        ubtt_bf=t_pack(Ubin[:, :H, :].astype(BF16)),
        ubbt_bf=t_pack(Ubin[:, H:, :].astype(BF16)),
        uunt_bf=t_pack(Uun.astype(BF16)),
        bd_bf=np.stack([b_rows, d_rows]).astype(BF16).reshape(2, 5 * H),
    )


def kernel(tokens, arity, emb, W, bW, Ubin, bUbin, Uun, bUun):
    from concourse.bass_utils import run_bass_kernel_spmd

    tokens = np.asarray(tokens)
    arity = np.asarray(arity)

    shared = prep_shared_inputs(emb, W, bW, Ubin, bUbin, Uun, bUun)
    in_maps = [
        prep_core_inputs(
            tokens[k * BL : (k + 1) * BL], arity[k * BL : (k + 1) * BL], shared
        )
        for k in range(NCORES)
    ]

    nc = _build_nc()
    res = run_bass_kernel_spmd(nc, in_maps, core_ids=list(range(NCORES)))
    results = res.results

    h = np.concatenate([r["h_out"].T for r in results], axis=0)
    c = np.concatenate([r["c_out"].T for r in results], axis=0)
    return h.astype(np.float32), c.astype(np.float32)


# revision 23
# speedup vs baseline: 1.0029x; 1.0029x over previous
"""MixedArityTreeLSTM Trainium2 kernel.

Level-synchronous bottom-up Tree-LSTM over B=256 heap-indexed perfect binary
trees (511 nodes, depth 8), E=H=128. Pure data-parallel over 8 NeuronCores
(32 trees per core); all weights replicated.

v4: host packs token-dependent data as dense feature-major bf16 streams (pure
data movement; arithmetic stays on device or is a vocab-indexed weight-only
table transform): hleaf = rows of tanh(emb @ W3 + bW3); xT = emb rows.

Every level is stored in BIT-REVERSED node order ("parity layout"), trees
fastest: left children of a level's positions [a, b) sit at the child level's
positions [a, b) and right children at [HALF + a, HALF + b). All child-pair
views are contiguous, which keeps the DVE on its fast 16-bit path.

Device per internal level (feature-major [H(part), nodes(free)]):
    pre_g = W_g^T x + Ubt_g^T (m*h_l) + Ubb_g^T (m*h_r) + Uun_g^T ((1-m)*h_l)
            + [b_g; d_g]^T [ones; m]      (K=2 matmul: bias + arity delta)
Gate pairs share PSUM tiles so one activation covers two gates. The unary
f_r kill uses cob = m * c_r. All elementwise work is bf16 in SBUF; c is bf16
except the root level. Emission is software-pipelined (masks/body/chain) and
levels <=4 split into two position-halves that ping-pong.
"""

import numpy as np
import ml_dtypes

B, D = 256, 8
V, E, H = 32000, 128, 128
NCORES = 8
BL = B // NCORES  # 32 trees per core

LVL_N = {l: BL * (2**l) for l in range(D + 1)}
INT_LEVELS = list(range(D - 1, -1, -1))  # 7..0
LVL_OFF = {}
_off = 0
for _l in INT_LEVELS:
    LVL_OFF[_l] = _off
    _off += LVL_N[_l]
XCOLS = _off  # 8160
LEAF_COLS = LVL_N[D]  # 8192

# bit-reversal position->node order per level: sig[l][i] = node at position i
SIG = {0: np.array([0])}
for _l in range(1, D + 1):
    SIG[_l] = np.concatenate([2 * SIG[_l - 1], 2 * SIG[_l - 1] + 1])

CPL = {8: 16, 7: 8, 6: 4, 5: 2, 4: 1, 3: 2, 2: 1, 1: 1, 0: 1}
CW = {l: LVL_N[l] // CPL[l] for l in range(D + 1)}

BIG_SEQ = [
    (7, 0), (7, 4), (6, 0), (7, 1), (7, 5), (6, 1), (7, 2),
    (7, 6), (6, 2), (5, 0), (7, 3), (7, 7), (6, 3), (5, 1),
]
TAIL_SEQ = [(l, j) for l in (4, 3, 2, 1, 0) for j in range(CPL[l])]
SEQ = BIG_SEQ + TAIL_SEQ


def _children(lvl, j):
    """Child chunks (lvl+1, jj) whose h/c this chunk consumes (parity layout)."""
    if lvl == D - 1:
        return []  # children are leaves (DMA'd, no chain)
    N = CW[lvl]
    c0 = j * N
    half = LVL_N[lvl]
    spans = [(c0, c0 + N), (half + c0, half + c0 + N)]
    out = []
    for jj in range(CPL[lvl + 1]):
        a, b = jj * CW[lvl + 1], (jj + 1) * CW[lvl + 1]
        if any(a < hi and b > lo for lo, hi in spans) and (lvl + 1, jj) not in out:
            out.append((lvl + 1, jj))
    return out


BF16 = ml_dtypes.bfloat16

_CACHE = {}


def _build_nc():
    if "nc" in _CACHE:
        return _CACHE["nc"]

    from contextlib import ExitStack

    import concourse.mybir as mybir
    import concourse.tile as tile
    from concourse import bacc

    dt = mybir.dt
    AF = mybir.ActivationFunctionType

    nc = bacc.Bacc()

    hleaf_d = nc.dram_tensor("hleaf", [128, LEAF_COLS], dt.bfloat16, kind="ExternalInput")
    xall_d = nc.dram_tensor("xall", [128, XCOLS], dt.bfloat16, kind="ExternalInput")
    mbc_d = nc.dram_tensor("mbcast", [128, XCOLS], dt.bfloat16, kind="ExternalInput")
    mk2_d = nc.dram_tensor("mk2", [2, XCOLS], dt.bfloat16, kind="ExternalInput")
    # host-pretransposed weights: [E, gate, H] etc., contiguous uploads
    wt_d = nc.dram_tensor("wt_bf", [E, 4 * H], dt.bfloat16, kind="ExternalInput")
    ubtt_d = nc.dram_tensor("ubtt_bf", [H, 5 * H], dt.bfloat16, kind="ExternalInput")
    ubbt_d = nc.dram_tensor("ubbt_bf", [H, 5 * H], dt.bfloat16, kind="ExternalInput")
    uunt_d = nc.dram_tensor("uunt_bf", [H, 4 * H], dt.bfloat16, kind="ExternalInput")
    bd_d = nc.dram_tensor("bd_bf", [2, 5 * H], dt.bfloat16, kind="ExternalInput")

    h_out_d = nc.dram_tensor("h_out", [H, BL], dt.float32, kind="ExternalOutput")
    c_out_d = nc.dram_tensor("c_out", [H, BL], dt.float32, kind="ExternalOutput")

    with tile.TileContext(nc) as tc, ExitStack() as ctx:
        consts = ctx.enter_context(tc.tile_pool(name="consts", bufs=1))
        lev = ctx.enter_context(tc.tile_pool(name="lev", bufs=1))

        h_t = {}
        c_t = {}
        h_t[D] = lev.tile([H, LEAF_COLS], dt.bfloat16, name="h_leaf", tag="h_leaf")
        for lvl in INT_LEVELS:
            n = LVL_N[lvl]
            hdt = dt.float32 if lvl == 0 else dt.bfloat16
            h_t[lvl] = lev.tile([H, n], hdt, name=f"h_l{lvl}", tag=f"h_l{lvl}")
            c_t[lvl] = lev.tile([H, n], hdt, name=f"c_l{lvl}", tag=f"c_l{lvl}")

        xt = lev.tile([128, XCOLS], dt.bfloat16, name="xT", tag="xT")
        mbc_sb = lev.tile([128, XCOLS], dt.bfloat16, name="mbc", tag="mbc")
        mk2_sb = lev.tile([2, XCOLS], dt.bfloat16, name="mk2", tag="mk2")

        # SP queue: leaf h (alternating halves: chunk (7,j) needs leaf j and
        # 8+j) interleaved with x pieces, in first-need order.
        def sdma(tile_, dram, a, b):
            nc.sync.dma_start(out=tile_[:, a:b], in_=dram[:, a:b])

        def xdma(a, b):
            nc.sync.dma_start(out=xt[:, a:b], in_=xall_d[:, a:b])

        # x first: the hoisted W-pass runway consumes xs before leaf h
        xdma(0, 1024)                       # (7,0)/(7,1) x
        sdma(h_t[D], hleaf_d, 0, 1024)      # (7,0)/(7,1) left
        sdma(h_t[D], hleaf_d, 4096, 5120)   # (7,0)/(7,1) right
        xdma(2048, 3072)                    # (7,4)/(7,5) x
        sdma(h_t[D], hleaf_d, 2048, 3072)   # (7,4)/(7,5) left
        sdma(h_t[D], hleaf_d, 6144, 7168)   # (7,4)/(7,5) right
        xdma(1024, 2048)                    # (7,2)/(7,3) x
        sdma(h_t[D], hleaf_d, 1024, 2048)   # (7,2)/(7,3) left
        sdma(h_t[D], hleaf_d, 5120, 6144)   # (7,2)/(7,3) right
        xdma(3072, 4096)                    # (7,6)/(7,7) x
        sdma(h_t[D], hleaf_d, 3072, 4096)   # (7,6)/(7,7) left
        sdma(h_t[D], hleaf_d, 7168, 8192)   # (7,6)/(7,7) right
        xdma(4096, 6144)                    # L6 x
        xdma(6144, XCOLS)                   # L5 + tail x

        # Pool (gpsimd) queue: broadcast mask pieces in chunk-need order
        # (L7 bodies touch 0/2048/512/..., L6 4096+, L5 6144+, tail last)
        for a, b in (
            (0, 512), (2048, 2560), (4096, 4608),
            (512, 1024), (2560, 3072), (4608, 5120),
            (1024, 1536), (3072, 3584), (5120, 5632),
            (1536, 2048), (3584, 4096), (5632, 6144),
            (6144, 7168), (7168, XCOLS),
        ):
            nc.gpsimd.dma_start(out=mbc_sb[:, a:b], in_=mbc_d[:, a:b])

        # Act queue: weights (host-pretransposed, contiguous)
        w_sb = consts.tile([E, 4, H], dt.bfloat16)
        nc.scalar.dma_start(
            out=w_sb, in_=wt_d[:, :].rearrange("e (g h) -> e g h", g=4)
        )
        nc.scalar.dma_start(out=mk2_sb[:, 0:4096], in_=mk2_d[:, 0:4096])
        bd_sb = consts.tile([2, 5, H], dt.bfloat16)
        nc.scalar.dma_start(
            out=bd_sb, in_=bd_d[:, :].rearrange("k (g h) -> k g h", g=5)
        )
        nc.scalar.dma_start(out=mk2_sb[:, 4096:XCOLS], in_=mk2_d[:, 4096:XCOLS])
        ubt_sb = consts.tile([H, 5, H], dt.bfloat16)
        nc.scalar.dma_start(
            out=ubt_sb, in_=ubtt_d[:, :].rearrange("k (g h) -> k g h", g=5)
        )
        ubb_sb = consts.tile([H, 5, H], dt.bfloat16)
        nc.scalar.dma_start(
            out=ubb_sb, in_=ubbt_d[:, :].rearrange("k (g h) -> k g h", g=5)
        )
        uun_sb = consts.tile([H, 4, H], dt.bfloat16)
        nc.scalar.dma_start(
            out=uun_sb, in_=uunt_d[:, :].rearrange("k (g h) -> k g h", g=4)
        )

        # PSUM: big tags (5 banks) + tail-odd parity tags (3 banks) = 8 banks
        psum = ctx.enter_context(tc.tile_pool(name="psum", bufs=1, space="PSUM"))
        work = ctx.enter_context(tc.tile_pool(name="work", bufs=4))

        # PE p-state warmup: ~3.5us of dummy matmuls so the Tensor engine is
        # at max clock when the first real chunk issues. Reuses the tail-odd
        # psum tag (idle until the tail) and only depends on the w_sb upload.
        wflat = w_sb.rearrange("e g h -> e (g h)")
        warm = psum.tile([H, 512], dt.float32, tag="tU1", name="warm")
        for _ in range(16):
            nc.tensor.matmul(warm, w_sb[:, 0, :], wflat, start=True, stop=True)

        state = {}

        def phase_masks(lvl, j):
            N = CW[lvl]
            c0 = j * N
            half = LVL_N[lvl]
            moff = LVL_OFF[lvl] + c0
            top = lvl == D - 1
            hch = h_t[lvl + 1]
            h_e = hch[:, c0 : c0 + N]
            h_o = hch[:, half + c0 : half + c0 + N]
            mb = mbc_sb[:, moff : moff + N]

            heb = work.tile([128, N], dt.bfloat16, tag="heb", name="heb")
            nc.vector.tensor_mul(heb, h_e, mb)
            hob = work.tile([128, N], dt.bfloat16, tag="hob", name="hob")
            nc.vector.tensor_mul(hob, h_o, mb)
            heu = work.tile([128, N], dt.bfloat16, tag="heu", name="heu")
            nc.vector.tensor_sub(heu, h_e, heb)
            st = {"heb": heb, "hob": hob, "heu": heu}
            if not top:
                cch = c_t[lvl + 1]
                st["c_e"] = cch[:, c0 : c0 + N]
                cob = work.tile([128, N], dt.bfloat16, tag="cob", name="cob")
                nc.vector.tensor_mul(cob, cch[:, half + c0 : half + c0 + N], mb)
                st["cob"] = cob
            state[(lvl, j)] = st

        def phase_body(lvl, j):
            N = CW[lvl]
            c0 = j * N
            moff = LVL_OFF[lvl] + c0
            top = lvl == D - 1
            st = state[(lvl, j)]
            heb, hob, heu = st["heb"], st["hob"], st["heu"]
            xs = xt[:, moff : moff + N]
            mk = mk2_sb[:, moff : moff + N]
            odd = lvl == 3 and j == 1
            tagA = "tA1" if odd else "bgA"
            tagB = "tB1" if odd else "bgB"
            tagU = "tU1" if odd else "bgu"

            # gate table: (psum slice key, W idx, Ubin idx, Uun idx, bd idx)
            # u first: the chain's first op t1 = gi*gu consumes gu, so the
            # tanh act must not be last; (fr, o) close last (t3/hmul inputs)
            if top:
                gates = [("U", 3, 4, 3, 4), ("A0", 0, 0, 0, 0), ("A1", 2, 3, 2, 3)]
            else:
                gates = [
                    ("U", 3, 4, 3, 4),     # u
                    ("A0", 0, 0, 0, 0),    # i
                    ("A1", 1, 1, 1, 1),    # fl
                    ("B0", 1, 2, None, 2),  # fr
                    ("B1", 2, 3, 2, 3),    # o
                ]

            pA = psum.tile([H, 2 * N], dt.float32, tag=tagA, name=f"pA{lvl}_{j}")
            pB = (
                psum.tile([H, 2 * N], dt.float32, tag=tagB, name=f"pB{lvl}_{j}")
                if not top
                else None
            )
            pU = psum.tile([H, N], dt.float32, tag=tagU, name=f"pU{lvl}_{j}")
            sl = {
                "A0": pA[:, 0:N],
                "A1": pA[:, N : 2 * N],
                "U": pU,
            }
            if not top:
                sl["B0"] = pB[:, 0:N]
                sl["B1"] = pB[:, N : 2 * N]

            # Big chunks (bank-aligned slices): dependency-free runway first —
            # W*x and bias/delta passes need only xs/mk, absorbing the
            # previous level's chain latency. Tail chunks share PSUM zero
            # regions between gate slices, so they run gate-major (one open
            # accumulation group per region).
            hoist = N >= 512
            if hoist:
                for key, wi, ubi, uui, gi_ in gates:
                    nc.tensor.matmul(
                        sl[key], w_sb[:, wi, :], xs, start=True, stop=False
                    )
                for key, wi, ubi, uui, gi_ in gates:
                    nc.tensor.matmul(
                        sl[key], bd_sb[:, gi_, :], mk, start=False, stop=False
                    )
            for key, wi, ubi, uui, gi_ in gates:
                ps = sl[key]
                if not hoist:
                    nc.tensor.matmul(ps, w_sb[:, wi, :], xs, start=True, stop=False)
                    nc.tensor.matmul(
                        ps, bd_sb[:, gi_, :], mk, start=False, stop=False
                    )
                nc.tensor.matmul(ps, ubt_sb[:, ubi, :], heb, start=False, stop=False)
                nc.tensor.matmul(
                    ps, ubb_sb[:, ubi, :], hob, start=False, stop=uui is None
                )
                if uui is not None:
                    nc.tensor.matmul(ps, uun_sb[:, uui, :], heu, start=False, stop=True)
                if key == "A1":
                    gAB = work.tile([128, 2 * N], dt.bfloat16, tag="gAB", name="gAB")
                    nc.scalar.activation(gAB, pA, AF.Sigmoid)
                    if top:
                        st["gi"], st["go"] = gAB[:, 0:N], gAB[:, N : 2 * N]
                    else:
                        st["gi"], st["gfl"] = gAB[:, 0:N], gAB[:, N : 2 * N]
                elif key == "B1":
                    gFO = work.tile([128, 2 * N], dt.bfloat16, tag="gFO", name="gFO")
                    nc.scalar.activation(gFO, pB, AF.Sigmoid)
                    st["gfr"], st["go"] = gFO[:, 0:N], gFO[:, N : 2 * N]
                elif key == "U":
                    gu = work.tile([128, N], dt.bfloat16, tag="gu", name="gu")
                    nc.scalar.activation(gu, pU, AF.Tanh)
                    st["gu"] = gu

        def phase_chain(lvl, j):
            N = CW[lvl]
            c0 = j * N
            top = lvl == D - 1
            st = state.pop((lvl, j))
            cs = c_t[lvl][:, c0 : c0 + N]
            wdt = dt.float32 if lvl == 0 else dt.bfloat16
            if top:
                nc.vector.tensor_mul(cs, st["gi"], st["gu"])
            else:
                t1 = work.tile([128, N], wdt, tag="t1", name="t1")
                nc.vector.tensor_mul(t1, st["gi"], st["gu"])
                t2 = work.tile([128, N], wdt, tag="t2", name="t2")
                nc.vector.tensor_mul(t2, st["gfl"], st["c_e"])
                nc.vector.tensor_add(cs, t1, t2)
                t3 = work.tile([128, N], wdt, tag="t3", name="t3")
                nc.vector.tensor_mul(t3, st["gfr"], st["cob"])
                nc.vector.tensor_add(cs, cs, t3)
            tch = work.tile([128, N], wdt, tag="tch", name="tch")
            nc.scalar.activation(tch, cs, AF.Tanh)
            nc.vector.tensor_mul(h_t[lvl][:, c0 : c0 + N], st["go"], tch)

        pending = []
        for lvl, j in SEQ:
            for ch in _children(lvl, j):
                if ch in pending:
                    phase_chain(*ch)
                    pending.remove(ch)
            phase_masks(lvl, j)
            phase_body(lvl, j)
            pending.append((lvl, j))
            while len(pending) > 1:
                phase_chain(*pending.pop(0))
        for ch in pending:
            phase_chain(*ch)

        nc.sync.dma_start(out=h_out_d[:, :], in_=h_t[0][:, :BL])
        nc.sync.dma_start(out=c_out_d[:, :], in_=c_t[0][:, :BL])

    nc.finalize()
    _CACHE["nc"] = nc
    return nc


def prep_core_inputs(tokens_c, arity_c, shared):
    """Per-core input map: gather rows of precomputed vocab tables, pack masks.

    Each level is packed in bit-reversed node order, trees fastest
    (col = position * BL + tree).
    """
    tokens_c = np.asarray(tokens_c)
    arity_c = np.asarray(arity_c, np.int32)
    emb_bf = shared["_emb_bf"]
    hleaf_tab = shared["_hleaf_tab"]

    leaf_toks = tokens_c[:, (2**D - 1) + SIG[D]].T.reshape(-1)  # [8192] node-major
    hleaf = np.ascontiguousarray(hleaf_tab[leaf_toks].T)

    xcols = []
    mcols = []
    for lvl in INT_LEVELS:
        off = 2**lvl - 1
        toks = tokens_c[:, off + SIG[lvl]].T.reshape(-1)
        xcols.append(emb_bf[toks].T)
        mcols.append(
            (arity_c[:, off + SIG[lvl]].T.reshape(-1) == 1).astype(BF16)
        )
    xall = np.ascontiguousarray(np.concatenate(xcols, axis=1))  # [128, 8160]
    maskb = np.concatenate(mcols)[None, :]  # [1, 8160]
    mk2 = np.concatenate([np.ones_like(maskb), maskb], axis=0)

    out = {k: v for k, v in shared.items() if not k.startswith("_")}
    out.update(
        hleaf=hleaf,
        xall=xall,
        mk2=mk2,
        mbcast=np.broadcast_to(maskb, (128, XCOLS)).copy(),
    )
    return out


def prep_shared_inputs(emb, W, bW, Ubin, bUbin, Uun, bUun):
    emb = np.asarray(emb, np.float32)
    W = np.asarray(W, np.float32)
    bW = np.asarray(bW, np.float32)
    Ubin = np.asarray(Ubin, np.float32)
    bUbin = np.asarray(bUbin, np.float32)
    Uun = np.asarray(Uun, np.float32)
    bUun = np.asarray(bUun, np.float32)

    b_rows = np.stack(
        [
            bW[0] + bUun[0],      # i
            bW[1] + bUun[1],      # fl
            bW[1] + bUbin[2],     # fr (binary value; unary killed via cob)
            bW[2] + bUun[2],      # o
            bW[3] + bUun[3],      # u
        ]
    )
    d_rows = np.stack(
        [
            bUbin[0] - bUun[0],
            bUbin[1] - bUun[1],
            np.zeros(H, np.float32),
            bUbin[3] - bUun[2],
            bUbin[4] - bUun[3],
        ]
    )

    emb_bf = emb.astype(BF16)
    hleaf_tab = np.tanh(emb @ W[3] + bW[3]).astype(BF16)

    def t_pack(a):  # [G, K, H] -> [K, G*H] contiguous
        return np.ascontiguousarray(np.transpose(a, (1, 0, 2)).reshape(a.shape[1], -1))

    return dict(
        _emb_bf=emb_bf,
        _hleaf_tab=hleaf_tab,
        wt_bf=t_pack(W.astype(BF16)),
        ubtt_bf=t_pack(Ubin[:, :H, :].astype(BF16)),
        ubbt_bf=t_pack(Ubin[:, H:, :].astype(BF16)),
        uunt_bf=t_pack(Uun.astype(BF16)),
        bd_bf=np.stack([b_rows, d_rows]).astype(BF16).reshape(2, 5 * H),
    )


def kernel(tokens, arity, emb, W, bW, Ubin, bUbin, Uun, bUun):
    from concourse.bass_utils import run_bass_kernel_spmd

    tokens = np.asarray(tokens)
    arity = np.asarray(arity)

    shared = prep_shared_inputs(emb, W, bW, Ubin, bUbin, Uun, bUun)
    in_maps = [
        prep_core_inputs(
            tokens[k * BL : (k + 1) * BL], arity[k * BL : (k + 1) * BL], shared
        )
        for k in range(NCORES)
    ]

    nc = _build_nc()
    res = run_bass_kernel_spmd(nc, in_maps, core_ids=list(range(NCORES)))
    results = res.results

    h = np.concatenate([r["h_out"].T for r in results], axis=0)
    c = np.concatenate([r["c_out"].T for r in results], axis=0)
    return h.astype(np.float32), c.astype(np.float32)
